# revision 1
# baseline (speedup 1.0000x reference)
"""Bass/Tile TRN2 kernel for nn_AttentionANEWraperChannelsFirstWithCache.

Tensor-parallel over heads across 8 NeuronCores:
  - 28 q heads padded to 32 slots (4 per core; odd cores carry 1 zero dummy).
  - core c owns kv head c//2 (each kv head replicated on a core pair).
  - per core: q/k/v projections for own slots, RoPE, in-SBUF cache update
    (K cache transposed to [d, s] via DMA-xbar transpose), attention over the
    full 4096-row cache in [s, l] layout with slots processed in pairs
    (scores/exp at free dim 1024), softmax denominator accumulated on DVE
    with a single fp32 ones-matmul per slot, normalization broadcast on
    GPSIMD.
  - per-slot AllGather of head outputs overlapped with later attention;
    column-parallel o_proj (448 output rows per core) at the end. Host
    concatenates the 8 row shards.

Matmul operands are bf16 (fp32 PSUM accumulation); softmax stats and
normalization stay fp32.
"""

import math
import numpy as np

H, KV, HD, LI = 28, 4, 128, 5
S_MAX, D, L = 4096, 3584, 512
NCORES = 8
SLOTS = 4                  # head slots per core (28 real heads padded to 32)
OSH = D // NCORES          # 448 o_proj output rows per core
NT = D // 128              # 28 contraction tiles over hidden dim
ST = S_MAX // 128          # 32 s-tiles over the cache
SCALE = 1.0 / math.sqrt(HD)


def _head_of(core, slot):
    off = 4 * (core % 2) + slot
    if off >= 7:
        return None                      # dummy slot
    return (core // 2) * 7 + off


# o_proj accumulation order: pair-major (matches the per-pair AllGather),
# then core, then pair half. Slot 3 exists only on even cores.
GROUPS = [(0, 1), (2, 3)]
REAL_JC = [(j, c) for g in GROUPS for c in range(NCORES)
           for j in g if _head_of(c, j) is not None]


_prog_cache = {}


def _build(cp):
    import concourse.bass as bass
    import concourse.mybir as mybir
    import concourse.tile as tile
    from concourse import bacc
    from contextlib import ExitStack

    f32 = mybir.dt.float32
    bf = mybir.dt.bfloat16
    AF = mybir.ActivationFunctionType
    nc = bacc.Bacc("TRN2", target_bir_lowering=False, debug=False,
                   num_devices=NCORES)

    x_d = nc.dram_tensor("x", [D, L], bf, kind="ExternalInput")
    wqT_d = nc.dram_tensor("wqT", [D, SLOTS * HD], bf, kind="ExternalInput")
    wkT_d = nc.dram_tensor("wkT", [D, HD], bf, kind="ExternalInput")
    wvT_d = nc.dram_tensor("wvT", [D, HD], bf, kind="ExternalInput")
    kc_d = nc.dram_tensor("kcache", [S_MAX, HD], bf, kind="ExternalInput")
    vc_d = nc.dram_tensor("vcache", [S_MAX, HD], bf, kind="ExternalInput")
    trig_d = nc.dram_tensor("trig", [HD, 4, L], f32, kind="ExternalInput")
    bias_d = nc.dram_tensor("biases", [HD, 6], f32, kind="ExternalInput")
    idrot_d = nc.dram_tensor("idrot", [HD, 2, HD], bf, kind="ExternalInput")
    woT_d = nc.dram_tensor("woT", [H * HD, OSH], bf, kind="ExternalInput")
    out_d = nc.dram_tensor("out", [OSH, L], f32, kind="ExternalOutput")

    wt0 = cp // 128                      # first window s-tile
    wset = set(range(wt0, wt0 + L // 128))
    # contiguous cache s-tile ranges outside the update window
    cr = []
    start = None
    for st in range(ST + 1):
        if st < ST and st not in wset:
            if start is None:
                start = st
        else:
            if start is not None:
                cr.append((start, st))
                start = None

    with tile.TileContext(nc) as tc, ExitStack() as ctx:
        const = ctx.enter_context(tc.tile_pool(name="const", bufs=1))
        persist = ctx.enter_context(tc.tile_pool(name="persist", bufs=1))
        kvpool = ctx.enter_context(tc.tile_pool(name="kvpool", bufs=1))
        wopool = ctx.enter_context(tc.tile_pool(name="wopool", bufs=1))
        agpool = ctx.enter_context(tc.tile_pool(name="agpool", bufs=1))
        dram = ctx.enter_context(tc.tile_pool(name="dram", bufs=1, space="DRAM"))

        ag_in = [dram.tile([len(g) * HD, L], bf, tag=f"agin{gi}",
                           name=f"ag_in{gi}")
                 for gi, g in enumerate(GROUPS)]
        ag_out = [dram.tile([NCORES * len(g) * HD, L], bf, tag=f"agout{gi}",
                            name=f"ag_out{gi}", addr_space="Shared")
                  for gi, g in enumerate(GROUPS)]

        # persistent buffers
        K_T = kvpool.tile([128, S_MAX], bf, tag="kt", name="K_T")   # [d, s]
        v_sb = kvpool.tile([128, S_MAX], bf, tag="v", name="v_sb")  # [s, d] tiles
        qpair = persist.tile([128, 2, L], bf, tag="qp", name="qpair")
        q2 = persist.tile([128, L], bf, tag="q2", name="q2_sb")
        q3 = persist.tile([128, L], bf, tag="q3", name="q3_sb")
        q_dst = [qpair[:, 0, :], qpair[:, 1, :], q2[:], q3[:]]

        x_r = x_d.rearrange("(t p) l -> p t l", p=128)
        wk_r = wkT_d.rearrange("(t p) d -> p t d", p=128)
        wv_r = wvT_d.rearrange("(t p) d -> p t d", p=128)
        vc_r = vc_d.rearrange("(t p) d -> p t d", p=128)

        scopeA = ExitStack()
        with scopeA:
            xpool = scopeA.enter_context(tc.tile_pool(name="xpool", bufs=1))
            wqpool = scopeA.enter_context(tc.tile_pool(name="wqpool", bufs=6))
            tmppool = scopeA.enter_context(tc.tile_pool(name="tmppool", bufs=4))
            pp = scopeA.enter_context(tc.tile_pool(name="pp", bufs=1, space="PSUM"))

            # ---- q projections first: PE starts as soon as x0/wq0 land ----
            x_sb = xpool.tile([128, NT, L], bf, tag="x", name="x_sb")
            wk_sb = xpool.tile([128, NT, HD], bf, tag="wk", name="wk_sb")
            wv_sb = xpool.tile([128, NT, HD], bf, tag="wv", name="wv_sb")
            q_ps = [pp.tile([128, L], f32, tag=f"pq{j}", name=f"q_ps{j}")
                    for j in range(SLOTS)]
            k_ps = pp.tile([128, L], f32, tag="pk", name="k_ps")
            v_ps = pp.tile([128, L], f32, tag="pv", name="v_ps")

            for t in range(NT):
                nc.sync.dma_start(out=x_sb[:, t, :], in_=x_r[:, t, :])
                wqt = wqpool.tile([128, SLOTS * HD], bf, tag="wq", name=f"wqt{t}")
                nc.sync.dma_start(out=wqt[:], in_=wqT_d[t * 128:(t + 1) * 128, :])
                if t == 20:
                    # bulk loads queued behind the first few proj tiles
                    nc.sync.dma_start(out=wk_sb[:], in_=wk_r[:])
                    nc.sync.dma_start(out=wv_sb[:], in_=wv_r[:])
                    trig = const.tile([HD, 4, L], f32, tag="trig", name="trig")
                    nc.sync.dma_start(out=trig[:], in_=trig_d[:])
                    bia = const.tile([HD, 6], f32, tag="bia", name="bia")
                    nc.sync.dma_start(out=bia[:], in_=bias_d[:])
                    idrot = const.tile([HD, 2, HD], bf, tag="idrot", name="idrot")
                    nc.sync.dma_start(out=idrot[:], in_=idrot_d[:])
                    ones_bf = const.tile([128, 1], bf, tag="ones_bf", name="ones_bf")
                    nc.gpsimd.memset(ones_bf[:], 1.0)
                    onesr_bf = const.tile([1, 128], bf, tag="onesr_bf", name="onesr_bf")
                    nc.gpsimd.memset(onesr_bf[:], 1.0)
                first, last = t == 0, t == NT - 1
                for j in range(SLOTS):
                    nc.tensor.matmul(q_ps[j][:], lhsT=wqt[:, j * 128:(j + 1) * 128],
                                     rhs=x_sb[:, t, :], start=first, stop=last)
            for t in range(NT):
                nc.tensor.matmul(k_ps[:], lhsT=wk_sb[:, t, :], rhs=x_sb[:, t, :],
                                 start=(t == 0), stop=(t == NT - 1))
            for t in range(NT):
                nc.tensor.matmul(v_ps[:], lhsT=wv_sb[:, t, :], rhs=x_sb[:, t, :],
                                 start=(t == 0), stop=(t == NT - 1))

            # ---- K cache -> K_T via DMA-xbar transpose; V cache straight ----
            for (a, b) in cr:
                nc.sync.dma_start_transpose(out=K_T[:, a * 128:b * 128],
                                            in_=kc_d[a * 128:b * 128, :])
                nc.sync.dma_start(out=v_sb[:, a * 128:b * 128],
                                  in_=vc_r[:, a:b, :])

            qcos, qsin = trig[:, 0, :], trig[:, 1, :]
            kcos, ksin = trig[:, 2, :], trig[:, 3, :]
            ident, rotm = idrot[:, 0, :], idrot[:, 1, :]

            # ---- bias + RoPE (rotate_half as a ±1 permutation matmul) ----
            def rope(dst, raw, cos_t, sin_t):
                rot_ps = pp.tile([128, L], f32, tag="tp", bufs=2, name="rot_ps")
                nc.tensor.matmul(rot_ps[:], lhsT=rotm, rhs=raw[:],
                                 start=True, stop=True)
                t1 = tmppool.tile([128, L], f32, tag="rt1", name="rt1")
                nc.vector.tensor_mul(t1[:], raw[:], cos_t)
                t2 = tmppool.tile([128, L], f32, tag="rt2", name="rt2")
                nc.vector.tensor_mul(t2[:], rot_ps[:], sin_t)
                nc.vector.tensor_add(dst, t1[:], t2[:])

            for j in range(SLOTS):
                q_raw = tmppool.tile([128, L], bf, tag="qraw", bufs=2, name=f"q_raw{j}")
                nc.scalar.activation(q_raw[:], q_ps[j][:], AF.Identity,
                                     bias=bia[:, j:j + 1])
                rope(q_dst[j], q_raw, qcos, qsin)

            k_raw = tmppool.tile([128, L], bf, tag="kraw", bufs=1, name="k_raw")
            nc.scalar.activation(k_raw[:], k_ps[:], AF.Identity, bias=bia[:, 4:5])
            rope(K_T[:, cp:cp + L], k_raw, kcos, ksin)

            v_raw = tmppool.tile([128, L], bf, tag="vraw", bufs=1, name="v_raw")
            nc.scalar.activation(v_raw[:], v_ps[:], AF.Identity, bias=bia[:, 5:6])
            for lt in range(L // 128):
                tp = pp.tile([128, 128], bf, tag="tp", bufs=2, name=f"tpv{lt}")
                nc.tensor.transpose(tp[:], v_raw[:, lt * 128:(lt + 1) * 128], ident)
                nc.scalar.copy(v_sb[:, (wt0 + lt) * 128:(wt0 + lt + 1) * 128], tp[:])

        # ---- o_proj weights prefetch (queued after phase-A DMAs) ----
        woT_sb = []
        for gi in range(len(REAL_JC)):
            w = wopool.tile([128, OSH], bf, name=f"woT{gi}")
            nc.sync.dma_start(out=w[:], in_=woT_d[gi * 128:(gi + 1) * 128, :])
            woT_sb.append(w)

        # ---- attention by gather groups; den folded on DVE (all-bf16 MMs) ----
        attg = {}
        scopeB = ExitStack()
        with scopeB:
            pa = scopeB.enter_context(tc.tile_pool(name="pa", bufs=1, space="PSUM"))
            ppool = scopeB.enter_context(tc.tile_pool(name="ppool", bufs=5))
            accpool = scopeB.enter_context(tc.tile_pool(name="accpool", bufs=1))
            spool = scopeB.enter_context(tc.tile_pool(name="spool", bufs=2))

            def make_tail(gi, slots_g, outs, accs):
                def tail():
                    for h, j in enumerate(slots_g):
                        acc_bf = ppool.tile([128, L], bf, tag="accbf", bufs=2,
                                            name=f"acc_bf{j}")
                        nc.vector.tensor_copy(acc_bf[:], accs[h][:])
                        den_ps = pa.tile([1, L], f32, tag="sc", bufs=2,
                                         name=f"den{j}")
                        nc.tensor.matmul(den_ps[:], lhsT=ones_bf[:],
                                         rhs=acc_bf[:], start=True, stop=True)
                        den_sb = spool.tile([1, L], f32, tag="den_sb",
                                            name=f"den_sb{j}")
                        nc.vector.tensor_copy(den_sb[:], den_ps[:])
                        rec = spool.tile([1, L], f32, tag="rec", name=f"rec{j}")
                        scr = spool.tile([1, L], f32, tag="scr", name=f"scr{j}")
                        nc.vector.reciprocal_approx_accurate(rec[:], den_sb[:],
                                                             scr[:])
                        rec_bf = spool.tile([1, L], bf, tag="rec_bf",
                                            name=f"rec_bf{j}")
                        nc.vector.tensor_copy(rec_bf[:], rec[:])
                        bc_ps = pa.tile([128, L], f32, tag="sc", bufs=2,
                                        name=f"bc_ps{j}")
                        nc.tensor.matmul(bc_ps[:], lhsT=onesr_bf[:],
                                         rhs=rec_bf[:], start=True, stop=True)
                        bc_sb = spool.tile([128, L], f32, tag="bc_sb",
                                           name=f"bc_sb{j}")
                        nc.vector.tensor_copy(bc_sb[:], bc_ps[:])
                        att = spool.tile([128, L], bf, tag=f"att{j}", bufs=1,
                                         name=f"att{j}")
                        nc.vector.tensor_mul(att[:], outs[h][:], bc_sb[:])
                        nc.sync.dma_start(out=ag_in[gi][h * HD:(h + 1) * HD, :],
                                          in_=att[:])
                    nc.gpsimd.collective_compute(
                        "AllGather",
                        mybir.AluOpType.bypass,
                        replica_groups=[list(range(NCORES))],
                        ins=[ag_in[gi].opt()],
                        outs=[ag_out[gi].opt()],
                    )
                    nh = len(slots_g)
                    agv = ag_out[gi].rearrange("(c h p) l -> p c h l",
                                               c=NCORES, h=nh, p=128)
                    ag_t = agpool.tile([128, NCORES, nh, L], bf,
                                       tag=f"attg{gi}", name=f"attg{gi}")
                    hc = NCORES // 2
                    nc.sync.dma_start(out=ag_t[:, 0:hc], in_=agv[:, 0:hc])
                    nc.sync.dma_start(out=ag_t[:, hc:], in_=agv[:, hc:])
                    attg[gi] = ag_t
                return tail

            q_src = {0: qpair[:, 0, :], 1: qpair[:, 1, :], 2: q2[:], 3: q3[:]}
            pending = []
            for gi, slots_g in enumerate(GROUPS):
                nh = len(slots_g)
                outs = [pa.tile([128, L], f32, tag=f"out{h}", bufs=2,
                                name=f"out{gi}_{h}") for h in range(nh)]
                accs = [accpool.tile([128, L], f32, tag=f"acc{j}",
                                     name=f"acc{j}") for j in slots_g]
                p_prev = None
                for st in range(ST):
                    if st == 2 and pending:
                        pending.pop(0)()
                    sc = pa.tile([128, nh * L], f32, tag="sc", bufs=2,
                                 name=f"sc{gi}_{st}")
                    kt = K_T[:, st * 128:(st + 1) * 128]
                    for h in range(nh):
                        nc.tensor.matmul(sc[:, h * L:(h + 1) * L], lhsT=kt,
                                         rhs=q_src[slots_g[h]],
                                         start=True, stop=True)
                    p = ppool.tile([128, nh * L], bf, tag="p", name=f"p{gi}_{st}")
                    nc.scalar.activation(p[:], sc[:], AF.Exp, scale=SCALE)
                    vt = v_sb[:, st * 128:(st + 1) * 128]
                    for h in range(nh):
                        nc.tensor.matmul(outs[h][:], lhsT=vt,
                                         rhs=p[:, h * L:(h + 1) * L],
                                         start=(st == 0), stop=(st == ST - 1))
                    if st % 2 == 0:
                        p_prev = p
                    else:
                        for h in range(nh):
                            tb = ppool.tile([128, L], bf, tag="tb", bufs=4,
                                            name=f"tb{gi}_{st}_{h}")
                            nc.vector.tensor_add(tb[:],
                                                 p_prev[:, h * L:(h + 1) * L],
                                                 p[:, h * L:(h + 1) * L])
                            if st == 1:
                                nc.vector.tensor_copy(accs[h][:], tb[:])
                            else:
                                nc.vector.tensor_add(accs[h][:], accs[h][:],
                                                     tb[:])
                pending.append(make_tail(gi, slots_g, outs, accs))
            for t_ in pending:
                t_()

        # ---- o_proj over gathered groups (PSUM banks reused) ----
        scopeC = ExitStack()
        with scopeC:
            po = scopeC.enter_context(tc.tile_pool(name="po", bufs=1, space="PSUM"))
            opool = scopeC.enter_context(tc.tile_pool(name="opool", bufs=2))

            o_ps = [po.tile([OSH // 4, L], f32, tag=f"o{ot}", name=f"o_ps{ot}")
                    for ot in range(4)]
            gi_ = 0
            NREAL = len(REAL_JC)
            for gidx, slots_g in enumerate(GROUPS):
                for c in range(NCORES):
                    for h, j in enumerate(slots_g):
                        if _head_of(c, j) is None:
                            continue
                        for ot in range(4):
                            m0 = ot * (OSH // 4)
                            nc.tensor.matmul(
                                o_ps[ot][:],
                                lhsT=woT_sb[gi_][:, m0:m0 + OSH // 4],
                                rhs=attg[gidx][:, c, h, :],
                                start=(gi_ == 0), stop=(gi_ == NREAL - 1))
                        gi_ += 1

            for ot in range(4):
                m0 = ot * (OSH // 4)
                osb = opool.tile([OSH // 4, L], f32, tag="osb", name=f"osb{ot}")
                nc.scalar.copy(osb[:], o_ps[ot][:])
                nc.sync.dma_start(out=out_d[m0:m0 + OSH // 4, :], in_=osb[:])

    nc.compile()
    return nc


def _get_prog(cp):
    if cp not in _prog_cache:
        _prog_cache[cp] = _build(cp)
    return _prog_cache[cp]


def _shards(hidden_states, cos, sin, cos_t, sin_t, key_cache, value_cache,
            wq, bq, wk, bk, wv, bv, wo):
    import ml_dtypes
    f = np.float32
    b16 = ml_dtypes.bfloat16
    x = np.ascontiguousarray(hidden_states.reshape(D, L)).astype(b16)
    qcos = np.asarray(cos_t, dtype=f).reshape(HD, L)
    qsin = np.asarray(sin_t, dtype=f).reshape(HD, L)
    kcos = np.asarray(cos, dtype=f).reshape(L, HD).T
    ksin = np.asarray(sin, dtype=f).reshape(L, HD).T
    trig = np.ascontiguousarray(np.stack([qcos, qsin, kcos, ksin], axis=1))
    rotm = np.zeros((HD, HD), dtype=f)   # rot(q) = R @ q; pass R.T as lhsT
    half = HD // 2
    rotm[np.arange(half), np.arange(half) + half] = -1.0
    rotm[np.arange(half) + half, np.arange(half)] = 1.0
    idrot = np.ascontiguousarray(
        np.stack([np.eye(HD, dtype=f), rotm.T], axis=1)).astype(b16)

    maps = []
    for c in range(NCORES):
        kvh = c // 2
        wqT = np.zeros((D, SLOTS * HD), dtype=f)
        biases = np.zeros((HD, 6), dtype=f)
        for s in range(SLOTS):
            h = _head_of(c, s)
            if h is None:
                continue
            wqT[:, s * HD:(s + 1) * HD] = wq[h * HD:(h + 1) * HD, :].T
            biases[:, s] = bq[h * HD:(h + 1) * HD]
        biases[:, 4] = bk[kvh * HD:(kvh + 1) * HD]
        biases[:, 5] = bv[kvh * HD:(kvh + 1) * HD]
        woT = np.empty((H * HD, OSH), dtype=f)
        rows = slice(OSH * c, OSH * (c + 1))
        for gi, (jj, cc) in enumerate(REAL_JC):
            h = _head_of(cc, jj)
            woT[gi * HD:(gi + 1) * HD, :] = wo[rows, h * HD:(h + 1) * HD].T
        maps.append({
            "x": x,
            "wqT": wqT.astype(b16),
            "wkT": np.ascontiguousarray(wk[kvh * HD:(kvh + 1) * HD, :].T).astype(b16),
            "wvT": np.ascontiguousarray(wv[kvh * HD:(kvh + 1) * HD, :].T).astype(b16),
            "kcache": np.ascontiguousarray(key_cache[LI, kvh]).astype(b16),
            "vcache": np.ascontiguousarray(value_cache[LI, kvh]).astype(b16),
            "trig": trig,
            "biases": np.ascontiguousarray(biases),
            "idrot": idrot,
            "woT": woT.astype(b16),
        })
    return maps


def kernel(_trace=False, **inputs):
    from concourse.bass_utils import run_bass_kernel_spmd

    cp = int(np.asarray(inputs["cache_position"]))
    assert cp % 128 == 0 and 0 <= cp <= S_MAX - L, f"unsupported cache_position {cp}"

    maps = _shards(
        inputs["hidden_states"], inputs["cos"], inputs["sin"],
        inputs["cos_t"], inputs["sin_t"],
        inputs["key_cache"], inputs["value_cache"],
        inputs["wq"], inputs["bq"], inputs["wk"], inputs["bk"],
        inputs["wv"], inputs["bv"], inputs["wo"],
    )
    nc = _get_prog(cp)
    res = run_bass_kernel_spmd(nc, maps, core_ids=list(range(NCORES)),
                               trace=_trace)
    out = np.concatenate([r["out"] for r in res.results], axis=0)
    out = out.astype(np.float32).reshape(1, D, 1, L)
    if _trace:
        return out, res
    return out



# revision 5
# speedup vs baseline: 1.0359x; 1.0359x over previous
"""Bass/Tile TRN2 kernel for nn_AttentionANEWraperChannelsFirstWithCache.

Tensor-parallel over heads across 8 NeuronCores (v2):
  - 28 q heads in 4 slots/core; core c owns kv head c//2 (replicated per pair).
  - Head-slot groups processed sequentially (slot 0,1,2,3), each over the full
    4096-row cache with the cache-update window tiles ordered last so
    attention starts before the k/v projections finish.
  - exp chunks of [128, 1024] (2 s-tiles) on the scalar engine, double
    buffered in PSUM; softmax denominator accumulated on DVE in bf16;
    per-slot biases applied on DVE (tensor_scalar_add).
  - AllGather per slot-pair {0,1} / slot {2} / slot {3}; the first two overlap
    later attention; o_proj accumulates in 2 rotating PSUM banks with DVE
    flushes into an SBUF accumulator, interleaved into slot-3 attention.
  - K cache pre-transposed on host ([d, s]); x/wq/wk/wv/v-cache/woT laid out
    host-side so every DMA is contiguous per partition.

Matmul operands bf16 (fp32 PSUM), softmax stats fp32/bf16 mix.
"""

import math
import numpy as np

H, KV, HD, LI = 28, 4, 128, 5
S_MAX, D, L = 4096, 3584, 512
NCORES = 8
SLOTS = 4
OSH = D // NCORES          # 448 o_proj output rows per core
NT = D // 128              # 28 contraction tiles over hidden dim
ST = S_MAX // 128          # 32 s-tiles over the cache
SCALE = 1.0 / math.sqrt(HD)


def _head_of(core, slot):
    off = 4 * (core % 2) + slot
    if off >= 7:
        return None                      # dummy slot (odd cores, slot 3)
    return (core // 2) * 7 + off


# o_proj entry order = gather-buffer order: group A (slots 0,1 on all cores),
# group B (slot 2 on all cores), group C (slot 3, even cores only).
ENTRIES = ([("A", c, h) for c in range(NCORES) for h in (0, 1)]
           + [("B", c, 2) for c in range(NCORES)]
           + [("C", c, 3) for c in range(0, NCORES, 2)])
assert len(ENTRIES) == H

_prog_cache = {}


def _build(cp):
    import concourse.bass as bass
    import concourse.mybir as mybir
    import concourse.tile as tile
    from concourse import bacc
    from contextlib import ExitStack

    f32 = mybir.dt.float32
    bf = mybir.dt.bfloat16
    AF = mybir.ActivationFunctionType
    nc = bacc.Bacc("TRN2", target_bir_lowering=False, debug=False,
                   num_devices=NCORES)

    x_d = nc.dram_tensor("x", [128, NT * L], bf, kind="ExternalInput")
    wq_d = nc.dram_tensor("wq", [SLOTS, 128, NT * 128], bf, kind="ExternalInput")
    wk_d = nc.dram_tensor("wk", [128, NT * 128], bf, kind="ExternalInput")
    wv_d = nc.dram_tensor("wv", [128, NT * 128], bf, kind="ExternalInput")
    kT_d = nc.dram_tensor("kT", [128, S_MAX], bf, kind="ExternalInput")
    v_d = nc.dram_tensor("v", [128, ST * 128], bf, kind="ExternalInput")
    trig_d = nc.dram_tensor("trig", [128, 4 * L], bf, kind="ExternalInput")
    bias_d = nc.dram_tensor("biases", [128, 6], f32, kind="ExternalInput")
    idrot_d = nc.dram_tensor("idrot", [128, 2 * 128], bf, kind="ExternalInput")
    wo_d = nc.dram_tensor("wo", [128, H * OSH], bf, kind="ExternalInput")
    out_d = nc.dram_tensor("out", [OSH, L], f32, kind="ExternalOutput")

    wt0 = cp // 128
    wset = set(range(wt0, wt0 + L // 128))
    # s-tile order: window (cache-update) tiles last
    SORD = [st for st in range(ST) if st not in wset] + sorted(wset)
    NCK = ST // 2                       # 16 chunks of 2 s-tiles per slot

    with tile.TileContext(nc) as tc, ExitStack() as ctx:
        const = ctx.enter_context(tc.tile_pool(name="const", bufs=1))
        persist = ctx.enter_context(tc.tile_pool(name="persist", bufs=1))
        kvpool = ctx.enter_context(tc.tile_pool(name="kvpool", bufs=1))
        wopool = ctx.enter_context(tc.tile_pool(name="wopool", bufs=1))
        agpool = ctx.enter_context(tc.tile_pool(name="agpool", bufs=1))
        spool = ctx.enter_context(tc.tile_pool(name="spool", bufs=2))
        ppool = ctx.enter_context(tc.tile_pool(name="ppool", bufs=4))
        accpool = ctx.enter_context(tc.tile_pool(name="accpool", bufs=2))
        pp = ctx.enter_context(tc.tile_pool(name="pp", bufs=1, space="PSUM"))
        dram = ctx.enter_context(tc.tile_pool(name="dram", bufs=1, space="DRAM"))

        ag_in = {g: dram.tile([nh * 128, L], bf, tag=f"agin{g}",
                              name=f"ag_in{g}")
                 for g, nh in (("A", 2), ("B", 1), ("C", 1))}
        ag_out = {g: dram.tile([NCORES * nh * 128, L], bf, tag=f"agout{g}",
                               name=f"ag_out{g}", addr_space="Shared")
                  for g, nh in (("A", 2), ("B", 1), ("C", 1))}

        # persistent SBUF
        K_T = kvpool.tile([128, S_MAX], bf, tag="kt", name="K_T")      # [d, s]
        v_sb = kvpool.tile([128, ST, 128], bf, tag="v", name="v_sb")   # [s,st,d]
        q_sb = [persist.tile([128, L], bf, tag=f"q{s}", name=f"q_sb{s}")
                for s in range(SLOTS)]
        osum = persist.tile([OSH // 4, 4, L], f32, tag="osum", name="osum")

        # ---- DMAs in priority order ----
        xw = ExitStack()
        xpool = xw.enter_context(tc.tile_pool(name="xpool", bufs=1))

        x_sb = xpool.tile([128, NT, L], bf, tag="x", name="x_sb")
        nc.sync.dma_start(out=x_sb[:], in_=x_d.rearrange("p (t l) -> p t l", l=L))
        wq_sb = []
        for s in range(SLOTS):
            w = xpool.tile([128, NT, 128], bf, tag=f"wq{s}", name=f"wq_sb{s}")
            wq_sb.append(w)
        nc.sync.dma_start(out=wq_sb[0][:],
                          in_=wq_d[0].rearrange("p (t d) -> p t d", d=128))
        trig = const.tile([128, 4, L], bf, tag="trig", name="trig")
        nc.sync.dma_start(out=trig[:], in_=trig_d.rearrange("p (i l) -> p i l", l=L))
        bia = const.tile([128, 6], f32, tag="bia", name="bia")
        nc.sync.dma_start(out=bia[:], in_=bias_d[:])
        idrot = const.tile([128, 2, 128], bf, tag="idrot", name="idrot")
        nc.sync.dma_start(out=idrot[:], in_=idrot_d.rearrange("p (i d) -> p i d", d=128))
        # K^T cache (non-window columns; host pre-transposed)
        nc.sync.dma_start(out=K_T[:, 0:cp], in_=kT_d[:, 0:cp])
        nc.sync.dma_start(out=K_T[:, cp + L:], in_=kT_d[:, cp + L:])
        # V cache (non-window s-tiles)
        v_r = v_d.rearrange("p (t d) -> p t d", d=128)
        nc.sync.dma_start(out=v_sb[:, 0:wt0], in_=v_r[:, 0:wt0])
        nc.sync.dma_start(out=v_sb[:, wt0 + 4:], in_=v_r[:, wt0 + 4:])
        wk_sb = xpool.tile([128, NT, 128], bf, tag="wk", name="wk_sb")
        nc.sync.dma_start(out=wk_sb[:], in_=wk_d.rearrange("p (t d) -> p t d", d=128))
        wv_sb = xpool.tile([128, NT, 128], bf, tag="wv", name="wv_sb")
        nc.sync.dma_start(out=wv_sb[:], in_=wv_d.rearrange("p (t d) -> p t d", d=128))
        for s in range(1, SLOTS):
            nc.sync.dma_start(out=wq_sb[s][:],
                              in_=wq_d[s].rearrange("p (t d) -> p t d", d=128))
        woT_sb = wopool.tile([128, H, OSH], bf, name="woT_sb")
        nc.sync.dma_start(out=woT_sb[:], in_=wo_d.rearrange("p (g o) -> p g o", o=OSH))

        ones_bf = const.tile([128, 1], bf, tag="ones_bf", name="ones_bf")
        nc.gpsimd.memset(ones_bf[:], 1.0)
        onesr_bf = const.tile([1, 128], bf, tag="onesr_bf", name="onesr_bf")
        nc.gpsimd.memset(onesr_bf[:], 1.0)

        qcos, qsin = trig[:, 0, :], trig[:, 1, :]
        kcos, ksin = trig[:, 2, :], trig[:, 3, :]
        ident, rotm = idrot[:, 0, :], idrot[:, 1, :]

        # ---- helpers ----
        def proj(w_sb, name):
            ps = pp.tile([128, L], f32, tag="op2", bufs=2, name=f"ps_{name}")
            for t in range(NT):
                nc.tensor.matmul(ps[:], lhsT=w_sb[:, t, :], rhs=x_sb[:, t, :],
                                 start=(t == 0), stop=(t == NT - 1))
            return ps

        def rope(dst, ps, bcol, cos_t, sin_t, name):
            raw = spool.tile([128, L], bf, tag="raw", name=f"raw_{name}")
            nc.vector.tensor_scalar_add(raw[:], ps[:], bia[:, bcol:bcol + 1])
            rot_ps = pp.tile([128, L], f32, tag="sc", bufs=2, name=f"rot_{name}")
            nc.tensor.matmul(rot_ps[:], lhsT=rotm, rhs=raw[:], start=True,
                             stop=True)
            t1 = spool.tile([128, L], bf, tag="rt1", name=f"rt1_{name}")
            nc.vector.tensor_mul(t1[:], raw[:], cos_t)
            t2 = spool.tile([128, L], bf, tag="rt2", name=f"rt2_{name}")
            nc.vector.tensor_mul(t2[:], rot_ps[:], sin_t)
            nc.vector.tensor_add(dst, t1[:], t2[:])

        # ---- projections for slot 0, then k/v queued as attention filler ----
        q_ps0 = proj(wq_sb[0], "q0")
        rope(q_sb[0][:], q_ps0, 0, qcos, qsin, "q0")

        def kv_fill():
            # generator: yields after small batches so attention interleaves
            ps_k = pp.tile([128, L], f32, tag="op2", bufs=2, name="ps_k")
            for t in range(NT):
                nc.tensor.matmul(ps_k[:], lhsT=wk_sb[:, t, :], rhs=x_sb[:, t, :],
                                 start=(t == 0), stop=(t == NT - 1))
                if t % 7 == 6:
                    yield
            rope(K_T[:, cp:cp + L], ps_k, 4, kcos, ksin, "k")
            yield
            ps_v = pp.tile([128, L], f32, tag="op2", bufs=2, name="ps_v")
            for t in range(NT):
                nc.tensor.matmul(ps_v[:], lhsT=wv_sb[:, t, :], rhs=x_sb[:, t, :],
                                 start=(t == 0), stop=(t == NT - 1))
                if t % 7 == 6:
                    yield
            v_raw = spool.tile([128, L], bf, tag="vraw", name="v_raw")
            nc.vector.tensor_scalar_add(v_raw[:], ps_v[:], bia[:, 5:6])
            for lt in range(4):
                tp = pp.tile([128, 128], bf, tag="sc", bufs=2, name=f"tpv{lt}")
                nc.tensor.transpose(tp[:], v_raw[:, lt * 128:(lt + 1) * 128],
                                    ident)
                nc.vector.tensor_copy(v_sb[:, wt0 + lt, :], tp[:])
            yield
            # slot 1..3 q projections, in small batches
            for s in range(1, SLOTS):
                ps_q = pp.tile([128, L], f32, tag="op2", bufs=2, name=f"ps_q{s}")
                for t in range(NT):
                    nc.tensor.matmul(ps_q[:], lhsT=wq_sb[s][:, t, :],
                                     rhs=x_sb[:, t, :],
                                     start=(t == 0), stop=(t == NT - 1))
                    if t % 7 == 6:
                        yield
                rope(q_sb[s][:], ps_q, s, qcos, qsin, f"q{s}")
                yield

        filler = [kv_fill()]

        def run_filler(n=1):
            for _ in range(n):
                if not filler:
                    return
                try:
                    next(filler[0])
                except StopIteration:
                    filler.pop(0)

        # ---- o_proj machinery (filled in during slot-3 attention) ----
        attg = {}

        def oproj_rounds(group, first, last):
            ents = [(gi, e) for gi, e in enumerate(ENTRIES) if e[0] == group]
            for ot in range(4):
                m0 = ot * (OSH // 4)
                bank = pp.tile([OSH // 4, L], f32, tag="op2", bufs=2,
                               name=f"ob_{group}{ot}")
                for i, (gi, e) in enumerate(ents):
                    g, c, h = e
                    if g == "A":
                        rhs = attg["A"][:, c, h, :]
                    else:
                        rhs = attg[g][:, c, :]
                    nc.tensor.matmul(bank[:],
                                     lhsT=woT_sb[:, gi, m0:m0 + OSH // 4],
                                     rhs=rhs,
                                     start=(i == 0), stop=(i == len(ents) - 1))
                    if i % 6 == 5:
                        yield
                if first:
                    nc.vector.tensor_copy(osum[:, ot, :], bank[:])
                else:
                    nc.vector.tensor_add(osum[:, ot, :], osum[:, ot, :],
                                         bank[:])
                yield
            if last:
                for ot in range(4):
                    m0 = ot * (OSH // 4)
                    nc.sync.dma_start(out=out_d[m0:m0 + OSH // 4, :],
                                      in_=osum[:, ot, :])

        def load_attg(g, nh):
            agv = ag_out[g].rearrange("(c h p) l -> p c h l", c=NCORES, h=nh,
                                      p=128)
            ag_t = agpool.tile([128, NCORES, nh, L], bf, tag=f"attg{g}",
                               name=f"attg{g}")
            hc = NCORES // 2
            nc.sync.dma_start(out=ag_t[:, 0:hc], in_=agv[:, 0:hc])
            nc.sync.dma_start(out=ag_t[:, hc:], in_=agv[:, hc:])
            if nh == 1:
                attg[g] = ag_t.rearrange("p c h l -> p (c h) l")
            else:
                attg[g] = ag_t

        # ---- attention: one slot at a time ----
        def tail(s, acc, out_ps, g, row, psum_tag):
            den_ps = pp.tile([1, L], f32, tag=psum_tag, bufs=2, name=f"den{s}")
            nc.tensor.matmul(den_ps[:], lhsT=ones_bf[:], rhs=acc[:, 0, :],
                             start=True, stop=False)
            nc.tensor.matmul(den_ps[:], lhsT=ones_bf[:], rhs=acc[:, 1, :],
                             start=False, stop=True)
            den_sb = spool.tile([1, L], f32, tag="den_sb", name=f"den_sb{s}")
            nc.vector.tensor_copy(den_sb[:], den_ps[:])
            rec = spool.tile([1, L], f32, tag="rec", name=f"rec{s}")
            scr = spool.tile([1, L], f32, tag="scr", name=f"scr{s}")
            nc.vector.reciprocal_approx_accurate(rec[:], den_sb[:], scr[:])
            rec_bf = spool.tile([1, L], bf, tag="rec_bf", name=f"rec_bf{s}")
            nc.vector.tensor_copy(rec_bf[:], rec[:])
            bc_ps = pp.tile([128, L], f32, tag=psum_tag, bufs=2,
                            name=f"bc{s}")
            nc.tensor.matmul(bc_ps[:], lhsT=onesr_bf[:], rhs=rec_bf[:],
                             start=True, stop=True)
            bc_sb = spool.tile([128, L], f32, tag="bc_sb", name=f"bc_sb{s}")
            nc.vector.tensor_copy(bc_sb[:], bc_ps[:])
            att = spool.tile([128, L], bf, tag="att", bufs=2, name=f"att{s}")
            nc.vector.tensor_mul(att[:], out_ps[:], bc_sb[:])
            nc.sync.dma_start(out=ag_in[g][row * 128:(row + 1) * 128, :],
                              in_=att[:])

        def gather(g, nh):
            nc.gpsimd.collective_compute(
                "AllGather",
                mybir.AluOpType.bypass,
                replica_groups=[list(range(NCORES))],
                ins=[ag_in[g].opt()],
                outs=[ag_out[g].opt()],
            )

        for s in range(SLOTS):
            acc = accpool.tile([128, 2, L], bf, tag="acc", name=f"acc{s}")
            out_ps = pp.tile([128, L], f32, tag="oab", bufs=2, name=f"out{s}")
            for ck in range(NCK):
                sa, sb = SORD[2 * ck], SORD[2 * ck + 1]
                sc = pp.tile([128, 2, L], f32, tag="sc", bufs=2,
                             name=f"sc{s}_{ck}")
                nc.tensor.matmul(sc[:, 0, :],
                                 lhsT=K_T[:, sa * 128:(sa + 1) * 128],
                                 rhs=q_sb[s][:], start=True, stop=True)
                nc.tensor.matmul(sc[:, 1, :],
                                 lhsT=K_T[:, sb * 128:(sb + 1) * 128],
                                 rhs=q_sb[s][:], start=True, stop=True)
                p = ppool.tile([128, 2, L], bf, tag="p", name=f"p{s}_{ck}")
                nc.scalar.activation(p[:], sc[:], AF.Exp, scale=SCALE)
                nc.tensor.matmul(out_ps[:], lhsT=v_sb[:, sa, :], rhs=p[:, 0, :],
                                 start=(ck == 0), stop=False)
                nc.tensor.matmul(out_ps[:], lhsT=v_sb[:, sb, :], rhs=p[:, 1, :],
                                 start=False, stop=(ck == NCK - 1))
                if ck == 0:
                    nc.vector.tensor_copy(acc[:], p[:])
                else:
                    nc.vector.tensor_add(acc[:], acc[:], p[:])
                run_filler(1)
            # group tails / gathers / o_proj interleave
            if s == 0:
                tail(s, acc, out_ps, "A", 0, "op2")
            elif s == 1:
                tail(s, acc, out_ps, "A", 1, "op2")
                gather("A", 2)
                load_attg("A", 2)
            elif s == 2:
                tail(s, acc, out_ps, "B", 0, "op2")
                gather("B", 1)
                load_attg("B", 1)
                filler.append(oproj_rounds("A", first=True, last=False))
            else:
                tail(s, acc, out_ps, "C", 0, "sc")
                gather("C", 1)
                load_attg("C", 1)

        # drain leftover filler, then remaining o_proj rounds
        while filler:
            run_filler(1)
        for _ in oproj_rounds("B", first=False, last=False):
            pass
        for _ in oproj_rounds("C", first=False, last=True):
            pass

        # exit x/wq scope (frees SBUF)
        xw.close()

    nc.compile()
    return nc


def _get_prog(cp):
    if cp not in _prog_cache:
        _prog_cache[cp] = _build(cp)
    return _prog_cache[cp]


def _shards(hidden_states, cos, sin, cos_t, sin_t, key_cache, value_cache,
            wq, bq, wk, bk, wv, bv, wo):
    import ml_dtypes
    f = np.float32
    b16 = ml_dtypes.bfloat16

    def tilemajor(wT):
        # [D, 128] (contraction-major) -> [128, NT*128] SBUF layout
        return np.ascontiguousarray(
            wT.reshape(NT, 128, -1).transpose(1, 0, 2).reshape(128, -1))

    x = hidden_states.reshape(D, L)
    x_arr = np.ascontiguousarray(
        x.reshape(NT, 128, L).transpose(1, 0, 2).reshape(128, NT * L)).astype(b16)
    qcos = np.asarray(cos_t, dtype=f).reshape(HD, L)
    qsin = np.asarray(sin_t, dtype=f).reshape(HD, L)
    kcos = np.asarray(cos, dtype=f).reshape(L, HD).T
    ksin = np.asarray(sin, dtype=f).reshape(L, HD).T
    trig = np.ascontiguousarray(
        np.concatenate([qcos, qsin, kcos, ksin], axis=1)).astype(b16)
    rotm = np.zeros((HD, HD), dtype=f)   # rot(q) = R @ q; pass R.T as lhsT
    half = HD // 2
    rotm[np.arange(half), np.arange(half) + half] = -1.0
    rotm[np.arange(half) + half, np.arange(half)] = 1.0
    idrot = np.ascontiguousarray(
        np.concatenate([np.eye(HD, dtype=f), rotm.T], axis=1)).astype(b16)

    maps = []
    for c in range(NCORES):
        kvh = c // 2
        wq_arr = np.zeros((SLOTS, 128, NT * 128), dtype=b16)
        biases = np.zeros((128, 6), dtype=f)
        for s in range(SLOTS):
            h = _head_of(c, s)
            if h is None:
                continue
            wq_arr[s] = tilemajor(
                np.ascontiguousarray(wq[h * HD:(h + 1) * HD, :].T)).astype(b16)
            biases[:, s] = bq[h * HD:(h + 1) * HD]
        biases[:, 4] = bk[kvh * HD:(kvh + 1) * HD]
        biases[:, 5] = bv[kvh * HD:(kvh + 1) * HD]
        kT = np.ascontiguousarray(key_cache[LI, kvh].T).astype(b16)
        vc = value_cache[LI, kvh]
        v_arr = np.ascontiguousarray(
            vc.reshape(ST, 128, HD).transpose(1, 0, 2).reshape(128, ST * HD)
        ).astype(b16)
        rows = slice(OSH * c, OSH * (c + 1))
        wo_arr = np.empty((128, H * OSH), dtype=b16)
        for gi, (g, cc, ss) in enumerate(ENTRIES):
            h = _head_of(cc, ss)
            wo_arr[:, gi * OSH:(gi + 1) * OSH] = \
                wo[rows, h * HD:(h + 1) * HD].T.astype(b16)
        maps.append({
            "x": x_arr,
            "wq": wq_arr,
            "wk": tilemajor(np.ascontiguousarray(
                wk[kvh * HD:(kvh + 1) * HD, :].T)).astype(b16),
            "wv": tilemajor(np.ascontiguousarray(
                wv[kvh * HD:(kvh + 1) * HD, :].T)).astype(b16),
            "kT": kT,
            "v": v_arr,
            "trig": trig,
            "biases": biases,
            "idrot": idrot,
            "wo": wo_arr,
        })
    return maps


def kernel(_trace=False, **inputs):
    from concourse.bass_utils import run_bass_kernel_spmd

    cp = int(np.asarray(inputs["cache_position"]))
    assert cp % 128 == 0 and 0 <= cp <= S_MAX - L, f"unsupported cache_position {cp}"

    maps = _shards(
        inputs["hidden_states"], inputs["cos"], inputs["sin"],
        inputs["cos_t"], inputs["sin_t"],
        inputs["key_cache"], inputs["value_cache"],
        inputs["wq"], inputs["bq"], inputs["wk"], inputs["bk"],
        inputs["wv"], inputs["bv"], inputs["wo"],
    )
    nc = _get_prog(cp)
    res = run_bass_kernel_spmd(nc, maps, core_ids=list(range(NCORES)),
                               trace=_trace)
    out = np.concatenate([r["out"] for r in res.results], axis=0)
    out = out.astype(np.float32).reshape(1, D, 1, L)
    if _trace:
        return out, res
    return out
